# revision 2
# baseline (speedup 1.0000x reference)
"""Biquad IIR filter (direct-form-II-transposed) on 8 Trainium2 NeuronCores.

V3.3: single-pass sliding-window FIR matmul (see V3), plus:

  - Stores carry all 128 partitions even though only P=114 hold real
    output: the DGE AP balancer splits a 114-partition DMA into
    19-partition groups -> only 6 of 16 SDMA engines carry it (~155
    GB/s store phase, the dominant tail in V3/V3.2/V3.3).  With 128
    partitions it splits 8/engine across all 16 (~414 GB/s).  The 14
    garbage partitions (+12% store bytes) are dropped on the host.

  - x loads in SINGLE rows, one semaphore per row waited at its final
    value: DMA-completion semaphores trail the last data byte by
    ~4us (write receipt + SDMA engine skew), and V3's row-PAIR gates
    stalled the PE 5.2us waiting for 2.6MB + lag.  Single rows halve
    the data wait per gate; the lag is paid once, not per row, since
    delivery (~3us/row) outpaces the PE row time.
  - NH=15 taps (worst-row truncation bound 5.8e-3 of output scale,
    actual much smaller - the bound assumes all tail taps align with
    max|x|), so P = 114 outputs/column and NCOL = 4608 window columns
    per row: total ring traffic drops 15.3 -> 14.3 MB/core, and the
    kernel is ring-bound (exec ~ 8.5us preamble + bytes/0.42 + tail).

Geometry: 9 chunks of 512 columns per row; evacuation units of 2
chunks (1024 cols) except the last unit of each row (512), 5 units
per row, ACT takes even global units, DVE odd, PSUM is 4 two-bank
tensors assigned per unit round-robin.

Kept from V2.x/V3: int8 y stores with per-row scale rounding to
nearest-even, ACT-table warm-up, partition-major DRAM layouts, waits
only at final semaphore values, HWDGE SP ring only, loads issued
before stores, no gpsimd drain.
"""

import sys

import numpy as np

if "/opt/trn_rl_repo" not in sys.path:
    sys.path.insert(0, "/opt/trn_rl_repo")

import concourse.bass as bass
import concourse.mybir as mybir
from concourse.bass_utils import run_bass_kernel_spmd

BATCH = 64
T = 524288
NCORES = 8
R = BATCH // NCORES  # rows per core
NH = 15  # FIR taps kept
M = 128  # contraction window
P = M - NH + 1  # 114 outputs per window column
NCOL = 4608  # window columns per row (114*4608 = 525312 >= T), 9 chunks
CHUNK = 512
NCH = NCOL // CHUNK  # 9 chunks per row
UPR = (NCH + 1) // 2  # evac units per row (5: four 1024s + one 512)
NPT = 4  # PSUM tensors (2 banks each)
SCREP = 16  # scale replication (512 B DMA lines)
WPAD = 128  # weight free-dim padded 114 -> 128 (2 KB DMA lines)
F16 = mybir.dt.float16
F32 = mybir.dt.float32
I8 = mybir.dt.int8

_CACHED = {}
ROUND_BIAS = 0.0  # fp32->int8 converts round-to-nearest-even (probed)


def _impulse_response(b: np.ndarray, a: np.ndarray, n: int) -> np.ndarray:
    """First n samples of the biquad impulse response, computed in f64."""
    nb = b.astype(np.float64)
    na = a.astype(np.float64)
    b0, b1, b2 = nb[:, 0], nb[:, 1], nb[:, 2]
    a1, a2 = na[:, 0], na[:, 1]
    rows = b.shape[0]
    h = np.zeros((rows, n), dtype=np.float64)
    z1 = np.zeros(rows, dtype=np.float64)
    z2 = np.zeros(rows, dtype=np.float64)
    for t in range(n):
        v0 = 1.0 if t == 0 else 0.0
        v1 = b0 * v0 + z1
        nz1 = b1 * v0 - a1 * v1 + z2
        nz2 = b2 * v0 - a2 * v1
        h[:, t] = v1
        z1, z2 = nz1, nz2
    return h


def _window_weights(h: np.ndarray) -> np.ndarray:
    """Per-row stationary operand W[r, k, i] = h[r, i + NH-1 - k],
    nonzero for i <= k <= i+NH-1; [rows, 128, P] padded to WPAD."""
    rows = h.shape[0]
    w = np.zeros((rows, M, WPAD), dtype=np.float64)
    for i in range(P):
        for d in range(NH):
            w[:, i + NH - 1 - d, i] = h[:, d]
    return w


# evac unit u (global, 5 per row): row u//UPR, unit j = u%UPR.
# j < 4: chunks 9r+2j, 9r+2j+1, cols 1024j..1024j+1024 (width 1024)
# j == 4: chunk 9r+8, cols 4096..4608 (width 512)
def _unit_cols(j):
    if j < UPR - 1:
        return 2 * CHUNK * j, 2 * CHUNK
    return 2 * CHUNK * (UPR - 1), CHUNK


def _unit_last_chunk(r, j):
    return NCH * r + (2 * j + 1 if j < UPR - 1 else NCH - 1)


def _eng_cnt(u):
    """(engine, count) pair for 'evac unit u done': ACT handles even
    units (its k-th instruction is unit 2(k-1)), DVE odd."""
    if u % 2 == 0:
        return "a", u // 2 + 1
    return "d", (u - 1) // 2 + 1


class _Waiter:
    """Emit a standalone wait_ge only when the target value increases."""

    def __init__(self, eng):
        self.eng = eng
        self.seen = {}

    def need(self, sem, val):
        if val <= 0:
            return
        if self.seen.get(sem.name, -1) >= val:
            return
        self.seen[sem.name] = val
        self.eng.wait_ge(sem, val)


def _build_bass(rows: int = R) -> bass.Bass:
    nc = bass.Bass(trn_type="TRN2")
    x_d = nc.declare_dram_parameter("x", [M, rows, NCOL], F16, isOutput=False)
    w_d = nc.declare_dram_parameter("w", [M, rows, WPAD], F16, isOutput=False)
    sc_d = nc.declare_dram_parameter("sc", [M, rows, SCREP], F32, isOutput=False)
    y_d = nc.declare_dram_parameter("y", [M, rows, NCOL], I8, isOutput=True)

    xt = nc.alloc_sbuf_tensor("xt_s", [M, rows, NCOL], F16).ap()
    ys = nc.alloc_sbuf_tensor("ys_s", [M, rows, NCOL], I8).ap()
    ws = nc.alloc_sbuf_tensor("ws_s", [M, rows, WPAD], F16).ap()
    sc = nc.alloc_sbuf_tensor("sc_s", [M, rows, SCREP], F32).ap()
    scr = nc.alloc_sbuf_tensor("scr_s", [M, 1], F32).ap()  # warmup scratch

    y_ps = [
        nc.alloc_psum_tensor(f"yps{i}", [M, 2 * CHUNK], F32).ap()
        for i in range(NPT)
    ]

    with (
        nc.Block(no_gpsimd_drain=True) as block,
        nc.semaphore("s_la") as s_la,
        nc.semaphore("s_lb") as s_lb,
        nc.semaphore("s_x1") as s_x1,
        nc.semaphore("s_x2") as s_x2,
        nc.semaphore("s_x3") as s_x3,
        nc.semaphore("s_x4") as s_x4,
        nc.semaphore("s_x5") as s_x5,
        nc.semaphore("s_x6") as s_x6,
        nc.semaphore("s_x7") as s_x7,
        nc.semaphore("s_mm") as s_mm,
        nc.semaphore("s_evd") as s_evd,
        nc.semaphore("s_eva") as s_eva,
        nc.semaphore("s_st") as s_st,
    ):
        row_sem = {1: s_x1, 2: s_x2, 3: s_x3, 4: s_x4,
                   5: s_x5, 6: s_x6, 7: s_x7}
        sem_of = {"a": s_eva, "d": s_evd}

        @block.sync
        def _(sp: bass.BassEngine):
            W = _Waiter(sp)
            # sc + w + row-0 first 4 chunks gate PE start (s_la, 48);
            # rest of row 0 on s_lb.  2048-col split keeps 4 KB lines.
            sp.dma_start(out=sc, in_=sc_d.ap()).then_inc(s_la, 16)
            sp.dma_start(out=ws, in_=w_d.ap()).then_inc(s_la, 16)
            HB = 4 * CHUNK  # 2048
            sp.dma_start(out=xt[:, 0, 0:HB], in_=x_d[:, 0, 0:HB]).then_inc(
                s_la, 16
            )
            sp.dma_start(out=xt[:, 0, HB:NCOL], in_=x_d[:, 0, HB:NCOL]).then_inc(
                s_lb, 16
            )
            for r, sem in row_sem.items():
                sp.dma_start(out=xt[:, r, :], in_=x_d[:, r, :]).then_inc(
                    sem, 16
                )
            # stores, all po2 lines.  Row r complete once units
            # 5r..5r+4 are evac'd (last two units cover both engines).
            def row_done(r):
                e1, c1 = _eng_cnt(UPR * (r + 1) - 1)
                e2, c2 = _eng_cnt(UPR * (r + 1) - 2)
                W.need(sem_of[e1], c1)
                W.need(sem_of[e2], c2)

            nst = 0
            for r0, r1 in ((0, 2), (2, 4), (4, 6)):
                row_done(r1 - 1)
                sp.dma_start(
                    out=y_d[:, r0:r1, :], in_=ys[:, r0:r1, :]
                ).then_inc(s_st, 16)
                nst += 1
            row_done(6)
            sp.dma_start(out=y_d[:, 6, :], in_=ys[:, 6, :]).then_inc(s_st, 16)
            # row 7 in halves so the last store (gated on the final
            # evacs) is small; first half needs units 35,36 only
            for e, c in (_eng_cnt(35), _eng_cnt(36)):
                W.need(sem_of[e], c)
            sp.dma_start(
                out=y_d[:, 7, 0:HB], in_=ys[:, 7, 0:HB]
            ).then_inc(s_st, 16)
            row_done(7)
            sp.dma_start(
                out=y_d[:, 7, HB:NCOL], in_=ys[:, 7, HB:NCOL]
            ).then_inc(s_st, 16)
            nst += 3
            W.need(s_st, 16 * nst)

        @block.scalar
        def _(a: bass.BassEngine):
            W = _Waiter(a)
            # pre-load the ACT function table during the DMA preamble
            a.activation(
                out=scr,
                in_=sc[:, 0, 0:1],
                func=mybir.ActivationFunctionType.Copy,
                bias=0.0,
                scale=1.0,
            )
            W.need(s_la, 48)
            for u in range(0, rows * UPR, 2):
                r, j = u // UPR, u % UPR
                c0, wdt = _unit_cols(j)
                W.need(s_mm, _unit_last_chunk(r, j) + 1)
                a.activation(
                    out=ys[:, r, c0 : c0 + wdt],
                    in_=y_ps[u % NPT][:, 0:wdt],
                    func=mybir.ActivationFunctionType.Copy,
                    bias=ROUND_BIAS,
                    scale=sc[:, r, 0:1],
                ).then_inc(s_eva, 1)

        @block.vector
        def _(v: bass.BassEngine):
            W = _Waiter(v)
            W.need(s_la, 48)
            for u in range(1, rows * UPR, 2):
                r, j = u // UPR, u % UPR
                c0, wdt = _unit_cols(j)
                W.need(s_mm, _unit_last_chunk(r, j) + 1)
                v.tensor_scalar(
                    ys[:, r, c0 : c0 + wdt],
                    y_ps[u % NPT][:, 0:wdt],
                    sc[:, r, 0:1],
                    ROUND_BIAS,
                    mybir.AluOpType.mult,
                    mybir.AluOpType.add,
                ).then_inc(s_evd, 1)

        @block.tensor
        def _(pe: bass.BassEngine):
            W = _Waiter(pe)
            W.need(s_la, 48)
            for r in range(rows):
                if r in row_sem:
                    W.need(row_sem[r], 16)
                for ch in range(NCH):
                    if r == 0 and ch == 4:
                        W.need(s_lb, 16)
                    u = UPR * r + min(ch // 2, UPR - 1)
                    slot = (ch - 2 * (u % UPR)) if ch < NCH - 1 else 0
                    bank = y_ps[u % NPT]
                    if u >= NPT and (ch % 2 == 0 or ch == NCH - 1):
                        # WAR: wait the evac that last used this tensor
                        e, c = _eng_cnt(u - NPT)
                        W.need(sem_of[e], c)
                    c0 = ch * CHUNK
                    nc.tensor.matmul(
                        bank[0:P, slot * CHUNK : (slot + 1) * CHUNK],
                        lhsT=ws[:, r, 0:P],
                        rhs=xt[:, r, c0 : c0 + CHUNK],
                        start=True,
                        stop=True,
                        skip_group_check=True,
                    ).then_inc(s_mm, 1)

    return nc


def _get_nc() -> bass.Bass:
    if "nc" not in _CACHED:
        _CACHED["nc"] = _build_bass()
    return _CACHED["nc"]


def run(x, b, a, trace=False, **spmd_kwargs):
    """Shard inputs, run the Bass kernel on 8 cores, gather full output."""
    assert x.shape == (BATCH, T), x.shape
    h = _impulse_response(b, a, NH)
    w = _window_weights(h).astype(np.float16)  # [BATCH, 128, WPAD]

    # per-row output scale: |y_r| <= ||h_r||_1 * max|x_r| (no saturation)
    h1 = np.abs(h).sum(axis=1)
    xmax = np.abs(x).max(axis=1).astype(np.float64)
    bound = h1 * xmax * 1.0001
    inv_s = np.float32(127.0 / bound)
    s_host = 1.0 / inv_s.astype(np.float64)

    # sliding-window layout: xw[k, r, f] = x[r, P f + k - (NH-1)]
    npad_tail = P * NCOL - T - (NH - 1)
    xpad = np.zeros((BATCH, NH - 1 + T + max(npad_tail, 0)), dtype=np.float16)
    xpad[:, NH - 1 : NH - 1 + T] = x.astype(np.float16)
    sv = np.lib.stride_tricks.as_strided(
        xpad,
        shape=(BATCH, NCOL, M),
        strides=(xpad.strides[0], P * xpad.strides[1], xpad.strides[1]),
    )
    xw = np.ascontiguousarray(sv.transpose(2, 0, 1))  # [M, BATCH, NCOL]

    sc_full = np.broadcast_to(
        inv_s[None, :, None], (M, BATCH, SCREP)
    ).astype(np.float32)
    in_maps = []
    for c in range(NCORES):
        rs = slice(c * R, (c + 1) * R)
        in_maps.append(
            {
                "x": np.ascontiguousarray(xw[:, rs]),
                "w": np.ascontiguousarray(w[rs].transpose(1, 0, 2)),
                "sc": np.ascontiguousarray(sc_full[:, rs]),
            }
        )
    nc = _get_nc()
    out = run_bass_kernel_spmd(
        nc, in_maps, list(range(NCORES)), trace=trace, **spmd_kwargs
    )
    # [P, rows, NCOL] int8 per core -> [BATCH, T] fp32 via per-row scale
    y_t = np.concatenate(
        [out.results[c]["y"] for c in range(NCORES)], axis=1
    )  # [M, BATCH, NCOL]; only the first P partitions are real output
    y = (
        y_t[:P].transpose(1, 2, 0).reshape(BATCH, NCOL * P)[:, :T]
        .astype(np.float32)
    )
    y *= s_host[:, None].astype(np.float32)
    return np.ascontiguousarray(y), out


def kernel(x, b, a):
    y, _ = run(x, b, a)
    return y
